# revision 1
# baseline (speedup 1.0000x reference)
"""Trainium2 Bass kernel for the DGL-JTNN tree-GRU encoder.

Math note: the reference runs a full up+down message-passing schedule, but the
output only reads h[ROOTS], and a root's in-edges are exactly the up-edges of
its two children.  Up-edge messages depend only on deeper up-edge messages, so
the entire down phase is dead code for the output.  We therefore compute only
the bottom-up pass, level by level over the balanced binary trees.

Layout: everything on-chip is feature-major [H=128 partitions, nodes], with
nodes ordered (tree-major, heap-order within the level).  In that order the
two children of parent column j at level d are columns 2j, 2j+1 of level d+1,
so all graph gathers become stride-2 adds / column-repeat broadcast APs.

Sharding: data-parallel over trees, 8 trees per NeuronCore; the small weight
matrices and the embedding table are replicated (as per the sharding hint).
"""

import os
import sys

import numpy as np

for _p in ("/opt/trn_rl_repo",):
    if os.path.isdir(_p) and _p not in sys.path:
        sys.path.insert(0, _p)

B, DEPTH, H, VOCAB = 64, 10, 128, 780
NPT = 2 ** (DEPTH + 1) - 1
NCORES = 8
T = B // NCORES  # trees per core
CHUNK = 512  # elementwise/ACT chunk width
MMN = 512  # max fp32 matmul moving dim
SCHUNK = 512  # small-level (single-chunk) tile width
GATHER_GROUP = int(os.environ.get("DGLJ_GG", "512"))  # embedding rows per dma_gather

NCOLS = {d: T * (1 << d) for d in range(DEPTH + 1)}
NPAD = {d: max(NCOLS[d], 128) for d in range(DEPTH + 1)}  # gather pad (idx %128)
IDX_OFF = {}
_off = 0
for _d in range(DEPTH, -1, -1):
    IDX_OFF[_d] = _off
    _off += NPAD[_d] // 16
GIDX_COLS = _off

_W_NAMES = ("wz1", "wz2", "wh1", "wh2", "wr", "ur", "wg1", "wg2")
_B_NAMES = ("bz", "bh", "br", "bg")
WPACK_COLS = 9 * H + len(_B_NAMES) + 1  # +1: negated bz for the zc trick

# float32r = native single-pass fp32 matmul mode (1 row/cycle at N>=256 vs 4
# for the exact 2-pass float32 path).  Validated on HW below; flip via env for
# experiments.
MM_F32R = os.environ.get("DGLJ_MM_F32R", "1") == "1"
SPLIT_TAIL = os.environ.get("DGLJ_SPLIT_TAIL", "1") == "1"
ZC_DVE = os.environ.get("DGLJ_ZC_DVE", "1") == "1"
U_POOL = os.environ.get("DGLJ_U_POOL", "1") == "1"

_NC_CACHE = {}


def _cdiv(a, b):
    return (a + b - 1) // b


def _build_nc(reps=1):
    """Build (and finalize) the per-core Bass program.  Same program for all 8
    cores; only the input data differs (SPMD)."""
    from contextlib import ExitStack

    import concourse.bass as bass
    import concourse.mybir as mybir
    import concourse.tile as tile
    from concourse import bacc

    f32 = mybir.dt.float32
    AF = mybir.ActivationFunctionType

    nc = bacc.Bacc("TRN2", target_bir_lowering=False)

    emb_d = nc.dram_tensor("emb", [VOCAB, H], f32, kind="ExternalInput")
    emb16_d = nc.dram_tensor(
        "emb16", [VOCAB, H], mybir.dt.bfloat16, kind="ExternalInput"
    )
    gidx_d = nc.dram_tensor(
        "gidx", [128, GIDX_COLS], mybir.dt.int16, kind="ExternalInput"
    )
    wpack_d = nc.dram_tensor("wpack", [H, WPACK_COLS], f32, kind="ExternalInput")
    out_d = nc.dram_tensor("out", [H, T], f32, kind="ExternalOutput")

    f32r = mybir.dt.float32r

    def mm(out, lhsT, rhs, start, stop):
        if MM_F32R:
            lhsT = lhsT.bitcast(f32r)
            rhs = rhs.bitcast(f32r)
        nc.tensor.matmul(out, lhsT, rhs, start=start, stop=stop)

    def mm16(out, lhsT, rhs, start, stop):
        nc.tensor.matmul(out, lhsT, rhs, start=start, stop=stop)

    def rnd(ap):
        # Output-AP cast so the producing op rounds to the f32r grid (the
        # verifier requires every fp32r-matmul operand to be pre-rounded).
        return ap.bitcast(f32r) if MM_F32R else ap

    with tile.TileContext(nc) as tc, ExitStack() as ctx:
        consts = ctx.enter_context(tc.tile_pool(name="consts", bufs=1))
        xpool = ctx.enter_context(tc.tile_pool(name="xp", bufs=1))
        mpool = ctx.enter_context(tc.tile_pool(name="mp", bufs=1))
        stpool = ctx.enter_context(tc.tile_pool(name="stage", bufs=3))
        ck = ctx.enter_context(tc.tile_pool(name="ck", bufs=2))
        pzp = ctx.enter_context(tc.tile_pool(name="pz", bufs=3, space="PSUM"))
        php = ctx.enter_context(tc.tile_pool(name="ph", bufs=2, space="PSUM"))
        prp = ctx.enter_context(tc.tile_pool(name="pr", bufs=2, space="PSUM"))
        pxp = ctx.enter_context(tc.tile_pool(name="px", bufs=1, space="PSUM"))

        # ---- constants into SBUF (gidx first: gathers depend on it) ----
        gidx = consts.tile([128, GIDX_COLS], mybir.dt.int16, tag="gidx", name="gidx")
        nc.sync.dma_start(out=gidx[:], in_=gidx_d[:])
        wld = consts.tile([H, WPACK_COLS], f32, tag="wld", name="wld")
        nc.sync.dma_start(out=wld[:], in_=wpack_d[:])
        wrnd = consts.tile([H, 8 * H], f32, tag="wrnd", name="wrnd")
        nc.vector.tensor_copy(rnd(wrnd[:]), wld[:, : 8 * H])
        wsb = {n: wrnd[:, i * H : (i + 1) * H] for i, n in enumerate(_W_NAMES)}
        bf16 = mybir.dt.bfloat16
        w16 = consts.tile([H, 4, H], bf16, tag="w16", name="w16")
        nc.vector.tensor_copy(
            w16[:],
            wld[:, : 8 * H].rearrange("p (w c) -> p w c", c=H)[:, 0::2, :],
        )
        wsb16 = {
            "wz1": w16[:, 0, :],
            "wh1": w16[:, 1, :],
            "wr": w16[:, 2, :],
            "wg1": w16[:, 3, :],
        }
        ident16 = consts.tile([H, H], bf16, tag="id16", name="ident16")
        nc.vector.tensor_copy(ident16[:], wld[:, 8 * H : 9 * H])
        # dummy 1-col sigmoid: hoists the ACT table load into the idle
        # startup window instead of serializing with the first real z
        warm = consts.tile([H, 1], f32, tag="warm", name="warm")
        nc.scalar.activation(warm[:], wld[:, :1], AF.Sigmoid)
        ident = wld[:, 8 * H : 9 * H]
        bsb = {n: wld[:, 9 * H + i : 9 * H + i + 1] for i, n in enumerate(_B_NAMES)}
        nbz = wld[:, 9 * H + 4 : 9 * H + 5]

        xt = {}  # level -> feature-major X tile [128, n_d]
        copy_alt = [0]  # alternate psum->sbuf copy engine

        def gather_level(d):
            """Gather level-d node embeddings (bf16) node-major, then PE
            transpose to feature-major [128, n]."""
            n = NCOLS[d]
            npad = NPAD[d]
            X = xpool.tile(
                [128, npad], mybir.dt.bfloat16, tag=f"x{d % 3}", name=f"x{d}"
            )
            xt[d] = X
            icol0 = IDX_OFF[d]
            # smaller first group at the leaf level so the first transposes
            # (and the whole pipeline) start sooner
            starts = list(range(0, npad, GATHER_GROUP))
            if d == DEPTH:
                starts = [0, 512] + [s + 512 for s in starts[1:-1]] + (
                    [npad - GATHER_GROUP + 512] if npad > GATHER_GROUP else []
                )
                starts = sorted(set(s for s in starts if s < npad))
            for si, i0 in enumerate(starts):
                nxt = starts[si + 1] if si + 1 < len(starts) else npad
                cnt = nxt - i0
                valid = cnt
                st = stpool.tile(
                    [128, GATHER_GROUP // 128, 128],
                    mybir.dt.bfloat16,
                    tag="stage",
                    name="st",
                )
                nc.gpsimd.dma_gather(
                    st[:, : _cdiv(cnt, 128), :],
                    emb16_d[:, :],
                    gidx[:, icol0 + i0 // 16 : icol0 + (i0 + cnt) // 16],
                    num_idxs=cnt,
                    num_idxs_reg=valid,
                    elem_size=H,
                )
                for p0 in range(0, valid, 1024):
                    w = min(1024, valid - p0)
                    pxt = pxp.tile([128, 1024], mybir.dt.bfloat16, tag="px", name="pxt")
                    for t0 in range(0, w, 128):
                        tw = min(128, w - t0)
                        ti = (p0 + t0) // 128
                        nc.tensor.transpose(
                            pxt[:, t0 : t0 + tw],
                            st[:tw, ti, :],
                            ident16[:tw, :tw],
                        )
                    dst = X[:, i0 + p0 : i0 + p0 + w]
                    nc.vector.tensor_copy(dst, pxt[:, :w])
                    copy_alt[0] += 1

        def level_small(d, M, RM, Mn, RMn, lo=0, w=None):
            # Latency-optimized single-chunk path: compute (1-z)*s off the
            # critical chain and feed Ur@m as Ur@a + Ur@(z*h~) so the
            # rm -> next-level chain is as short as possible.  [lo, lo+w) is a
            # tree-aligned column subrange: two half-batches pipeline the
            # otherwise serial level chain.
            n = NCOLS[d]
            if w is None:
                w = n
            X = xt[d]
            Xp = xt[d - 1]
            Mv = Mn[:, 2 * lo : 2 * (lo + w)].rearrange("p (n two) -> p n two", two=2)
            RMv = RMn[:, 2 * lo : 2 * (lo + w)].rearrange("p (n two) -> p n two", two=2)
            S = ck.tile([128, SCHUNK], f32, tag="s", name="S", bufs=3)
            nc.vector.tensor_add(rnd(S[:, :w]), Mv[:, :, 0], Mv[:, :, 1])
            zt = pzp.tile([128, SCHUNK], f32, tag="pz", name="zt")
            mm16(zt[:, :w], wsb16["wz1"], X[:, lo : lo + w], start=True, stop=False)
            mm(zt[:, :w], wsb["wz2"], S[:, :w], start=False, stop=True)
            z = ck.tile([128, SCHUNK], f32, tag="z", name="z", bufs=3)
            nc.scalar.activation(z[:, :w], zt[:, :w], AF.Sigmoid, bias=bsb["bz"])
            zc = ck.tile([128, SCHUNK], f32, tag="z", name="zc", bufs=3)
            if ZC_DVE:
                nc.vector.tensor_scalar(
                    out=zc[:, :w], in0=z[:, :w], scalar1=-1.0, scalar2=1.0,
                    op0=mybir.AluOpType.mult, op1=mybir.AluOpType.add,
                )
            else:
                nc.scalar.activation(
                    zc[:, :w], zt[:, :w], AF.Sigmoid, bias=nbz, scale=-1.0
                )
            a = ck.tile([128, SCHUNK], f32, tag="u", name="a", bufs=3)
            nc.gpsimd.tensor_mul(rnd(a[:, :w]), zc[:, :w], S[:, :w])
            last = d == 1
            if not last:
                rt = prp.tile([128, SCHUNK], f32, tag="pr", name="rt")
                mm(rt[:, :w], wsb["ur"], a[:, :w], start=True, stop=False)
                wp = w // 2
                xpb = Xp[:, lo // 2 : lo // 2 + wp].to_broadcast([128, wp, 2])
                mm16(rt[:, :w], wsb16["wr"], xpb, start=False, stop=False)
            htp = php.tile([128, SCHUNK], f32, tag="ph", name="htp")
            mm16(htp[:, :w], wsb16["wh1"], X[:, lo : lo + w], start=True, stop=False)
            mm(htp[:, :w], wsb["wh2"], RMv[:, :, 0], start=False, stop=False)
            mm(htp[:, :w], wsb["wh2"], RMv[:, :, 1], start=False, stop=True)
            ht = ck.tile([128, SCHUNK], f32, tag="h", name="ht", bufs=3)
            nc.scalar.activation(ht[:, :w], htp[:, :w], AF.Tanh, bias=bsb["bh"])
            t2 = ck.tile([128, SCHUNK], f32, tag="u", name="t2", bufs=3)
            nc.vector.tensor_mul(rnd(t2[:, :w]), z[:, :w], ht[:, :w])
            nc.vector.tensor_add(rnd(M[:, lo : lo + w]), a[:, :w], t2[:, :w])
            if not last:
                mm(rt[:, :w], wsb["ur"], t2[:, :w], start=False, stop=True)
                r = ck.tile([128, SCHUNK], f32, tag="h", name="r", bufs=3)
                nc.scalar.activation(r[:, :w], rt[:, :w], AF.Sigmoid, bias=bsb["br"])
                nc.vector.tensor_mul(rnd(RM[:, lo : lo + w]), r[:, :w], M[:, lo : lo + w])

        def level_compute(d, M, RM, Mn, RMn):
            n = NCOLS[d]
            if n <= SCHUNK and d < DEPTH:
                if SPLIT_TAIL and n >= 32:
                    level_small(d, M, RM, Mn, RMn, lo=0, w=n // 2)
                    level_small(d, M, RM, Mn, RMn, lo=n // 2, w=n // 2)
                else:
                    level_small(d, M, RM, Mn, RMn)
                return
            X = xt[d]
            Xp = xt[d - 1]
            for c0 in range(0, n, CHUNK):
                w = min(CHUNK, n - c0)
                cs = slice(c0, c0 + w)
                leaf = d == DEPTH
                halves = [(q0, min(MMN, w - q0)) for q0 in range(0, w, MMN)]
                if not leaf:
                    Mv = Mn[:, 2 * c0 : 2 * c0 + 2 * w].rearrange(
                        "p (n two) -> p n two", two=2
                    )
                    RMv = RMn[:, 2 * c0 : 2 * c0 + 2 * w].rearrange(
                        "p (n two) -> p n two", two=2
                    )
                    S = ck.tile([128, CHUNK], f32, tag="s", name="S", bufs=3)
                    nc.vector.tensor_add(rnd(S[:, :w]), Mv[:, :, 0], Mv[:, :, 1])
                # z = sigmoid(Wz1 @ x + Wz2 @ s + bz)   (feature-major preacts)
                zt = pzp.tile([128, CHUNK], f32, tag="pz", name="zt")
                for q0, qw in halves:
                    qs = slice(q0, q0 + qw)
                    mm16(zt[:, qs], wsb16["wz1"], X[:, c0 + q0 : c0 + q0 + qw],
                         start=True, stop=leaf)
                    if not leaf:
                        mm(zt[:, qs], wsb["wz2"], S[:, qs], start=False, stop=True)
                z = ck.tile([128, CHUNK], f32, tag="z", name="z", bufs=3)
                nc.scalar.activation(z[:, :w], zt[:, :w], AF.Sigmoid, bias=bsb["bz"])
                # h~ = tanh(Wh1 @ x + Wh2 @ arm + bh); arm pairsum folded into PSUM
                htp = php.tile([128, CHUNK], f32, tag="ph", name="htp")
                for q0, qw in halves:
                    qs = slice(q0, q0 + qw)
                    mm16(htp[:, qs], wsb16["wh1"], X[:, c0 + q0 : c0 + q0 + qw],
                         start=True, stop=leaf)
                    if not leaf:
                        mm(htp[:, qs], wsb["wh2"], RMv[:, q0 : q0 + qw, 0],
                           start=False, stop=False)
                        mm(htp[:, qs], wsb["wh2"], RMv[:, q0 : q0 + qw, 1],
                           start=False, stop=True)
                ht = ck.tile([128, CHUNK], f32, tag="h", name="ht", bufs=3)
                nc.scalar.activation(ht[:, :w], htp[:, :w], AF.Tanh, bias=bsb["bh"])
                # m = s + z * (h~ - s)    (leaf: m = z * h~)
                if leaf:
                    nc.vector.tensor_mul(rnd(M[:, cs]), z[:, :w], ht[:, :w])
                else:
                    u = ck.tile([128, CHUNK], f32, tag="u", name="u", bufs=3)
                    nc.vector.tensor_sub(u[:, :w], ht[:, :w], S[:, :w]) if not U_POOL else nc.gpsimd.tensor_sub(u[:, :w], ht[:, :w], S[:, :w])
                    v = ck.tile([128, CHUNK], f32, tag="v", name="v", bufs=3)
                    nc.vector.tensor_mul(v[:, :w], z[:, :w], u[:, :w])
                    nc.vector.tensor_add(rnd(M[:, cs]), S[:, :w], v[:, :w])
                if d == 1:
                    # rm of level 1 feeds nothing the output needs
                    continue
                # r = sigmoid(Wr @ x_parent + Ur @ m + br); parent cols repeat 2x
                rt = prp.tile([128, CHUNK], f32, tag="pr", name="rt")
                for q0, qw in halves:
                    qs = slice(q0, q0 + qw)
                    mm(rt[:, qs], wsb["ur"], M[:, c0 + q0 : c0 + q0 + qw],
                       start=True, stop=False)
                    qp = qw // 2
                    xpb = Xp[:, (c0 + q0) // 2 : (c0 + q0) // 2 + qp].to_broadcast(
                        [128, qp, 2]
                    )
                    mm16(rt[:, qs], wsb16["wr"], xpb, start=False, stop=True)
                r = ck.tile([128, CHUNK], f32, tag="r", name="r", bufs=3)
                nc.scalar.activation(r[:, :w], rt[:, :w], AF.Sigmoid, bias=bsb["br"])
                nc.gpsimd.tensor_mul(rnd(RM[:, cs]), r[:, :w], M[:, cs])

        # offsets of levels <= SMALL_X_MAX inside the combined xsmall tile
        SMALL_X_MAX = 7
        small_off = {}
        _o = 0
        for _d in range(SMALL_X_MAX, -1, -1):
            small_off[_d] = _o
            _o += NPAD[_d]
        SMALL_COLS = _o

        def gather_small():
            xs = xpool.tile(
                [128, SMALL_COLS], mybir.dt.bfloat16, tag="xs", name="xsmall"
            )
            for _d in range(SMALL_X_MAX, -1, -1):
                xt[_d] = xs[:, small_off[_d] : small_off[_d] + NPAD[_d]]
            icol0 = IDX_OFF[SMALL_X_MAX]
            for i0 in range(0, SMALL_COLS, GATHER_GROUP):
                cnt = min(GATHER_GROUP, SMALL_COLS - i0)
                st = stpool.tile(
                    [128, GATHER_GROUP // 128, 128],
                    mybir.dt.bfloat16,
                    tag="stage",
                    name="st",
                )
                nc.gpsimd.dma_gather(
                    st[:, : _cdiv(cnt, 128), :],
                    emb16_d[:, :],
                    gidx[:, icol0 + i0 // 16 : icol0 + (i0 + cnt) // 16],
                    num_idxs=cnt,
                    num_idxs_reg=cnt,
                    elem_size=H,
                )
                for p0 in range(0, cnt, 1024):
                    w = min(1024, cnt - p0)
                    pxt = pxp.tile([128, 1024], mybir.dt.bfloat16, tag="px", name="pxt")
                    for t0 in range(0, w, 128):
                        tw = min(128, w - t0)
                        ti = (p0 + t0) // 128
                        nc.tensor.transpose(
                            pxt[:, t0 : t0 + tw], st[:tw, ti, :], ident16[:tw, :tw]
                        )
                    nc.vector.tensor_copy(xs[:, i0 + p0 : i0 + p0 + w], pxt[:, :w])

        # ---- schedule ----
        for _rep in range(reps):
            gather_level(DEPTH)
            gather_level(DEPTH - 1)
            gather_small()
            Mn = RMn = None
            M1 = None
            for d in range(DEPTH, 0, -1):
                if DEPTH - 1 >= d - 2 >= 8:
                    gather_level(d - 2)
                M = mpool.tile([128, NCOLS[d]], f32, tag=f"m{d % 2}", name=f"M{d}")
                RM = None
                if d > 1:
                    RM = mpool.tile([128, NCOLS[d]], f32, tag=f"rm{d % 2}", name=f"RM{d}")
                level_compute(d, M, RM, Mn, RMn)
                Mn, RMn = M, RM
                if d == 1:
                    M1 = M
            # ---- root readout: relu(Wg1 @ x_root + Wg2 @ (m_c1 + m_c2) + bg)
            M1v = M1[:, : 2 * T].rearrange("p (n two) -> p n two", two=2)
            S1 = ck.tile([128, T], f32, tag="s", name="S1", bufs=3)
            nc.vector.tensor_add(rnd(S1[:]), M1v[:, :, 0], M1v[:, :, 1])
            pg = pzp.tile([128, T], f32, tag="pz", name="pg")
            mm16(pg[:], wsb16["wg1"], xt[0][:, :T], start=True, stop=False)
            mm(pg[:], wsb["wg2"], S1[:], start=False, stop=True)
            outt = ck.tile([128, T], f32, tag="h", name="outt", bufs=3)
            nc.scalar.activation(outt[:], pg[:], AF.Relu, bias=bsb["bg"])
            nc.sync.dma_start(out=out_d[:, :], in_=outt[:])

    nc.finalize()
    return nc


def get_nc(reps=1):
    key = ("nc", reps)
    if key not in _NC_CACHE:
        _NC_CACHE[key] = _build_nc(reps)
    return _NC_CACHE[key]


def make_core_inputs(wid, emb, weights):
    """Per-core input dicts.  `weights` is the dict of raw weight arrays."""
    wid = np.asarray(wid).reshape(B, NPT)
    wmats = {
        "wz1": weights["Wz_w"][:H],
        "wz2": weights["Wz_w"][H:],
        "wh1": weights["Wh_w"][:H],
        "wh2": weights["Wh_w"][H:],
        "wr": weights["Wr_w"],
        "ur": weights["Ur_w"],
        "wg1": weights["Wg_w"][:H],
        "wg2": weights["Wg_w"][H:],
    }
    bvecs = {
        "bz": weights["Wz_b"],
        "bh": weights["Wh_b"],
        "br": weights["Ur_b"],
        "bg": weights["Wg_b"],
    }
    wpack = np.zeros((H, WPACK_COLS), dtype=np.float32)
    for i, n in enumerate(_W_NAMES):
        wpack[:, i * H : (i + 1) * H] = wmats[n].astype(np.float32)
    wpack[:, 8 * H : 9 * H] = np.eye(H, dtype=np.float32)
    for i, n in enumerate(_B_NAMES):
        wpack[:, 9 * H + i] = bvecs[n].astype(np.float32)
    wpack[:, 9 * H + len(_B_NAMES)] = -bvecs["bz"].astype(np.float32)
    import ml_dtypes

    embf = np.ascontiguousarray(np.asarray(emb, dtype=np.float32))
    base = {
        "emb": embf,
        "emb16": np.ascontiguousarray(embf.astype(ml_dtypes.bfloat16)),
        "wpack": wpack,
    }
    in_maps = []
    for c in range(NCORES):
        widc = wid[c * T : (c + 1) * T]
        blocks = []
        for d in range(DEPTH, -1, -1):
            ids = widc[:, (1 << d) - 1 : (1 << (d + 1)) - 1].reshape(-1)
            ids = ids.astype(np.int16)
            pad = NPAD[d] - len(ids)
            if pad:
                ids = np.concatenate([ids, np.zeros(pad, np.int16)])
            blocks.append(ids.reshape(-1, 16).T)
        gi = np.concatenate(blocks, axis=1)  # [16, GIDX_COLS]
        assert gi.shape == (16, GIDX_COLS), gi.shape
        in_maps.append({**base, "gidx": np.ascontiguousarray(np.tile(gi, (8, 1)))})
    return in_maps


def kernel(**inputs):
    from concourse.bass_utils import run_bass_kernel_spmd

    nc = get_nc()
    in_maps = make_core_inputs(inputs["wid"], inputs["emb"], inputs)
    res = run_bass_kernel_spmd(nc, in_maps, core_ids=list(range(NCORES)))
    out = np.concatenate(
        [np.asarray(res.results[c]["out"]).T for c in range(NCORES)], axis=0
    )
    return np.ascontiguousarray(out.astype(np.float32))



# revision 14
# speedup vs baseline: 1.3793x; 1.3793x over previous
"""Trainium2 Bass kernel for the DGL-JTNN tree-GRU encoder (v3).

Math: only the bottom-up pass matters for the root readout (the down phase is
dead code for h[ROOTS]).  Beyond that, this version exploits the vocab
structure: x = emb[wid] has only 780 distinct values, so every x-only
quantity is a 780-row table computed once on the host:

  mtab[v]  = sigmoid(emb_v @ Wz1 + bz) * tanh(emb_v @ Wh1 + bh)   (leaf m)
  xtab[v]  = emb_v (fp16)                                         (interior x)

The whole leaf level (z, h~, m for 8192 edges/core) collapses into
dma_gathers of mtab rows.  All gathers use dma_gather(transpose=True), which
writes feature-major [128, n] tiles directly -- no PE transposes, no
PSUM->SBUF copies.

Layout: each level's columns use a BLOCK-LOCAL parity split: level d is
processed in chunks of K[d] parent columns; the 2*K children of a parent
chunk form one contiguous block of level d+1, even children in the first
half, odd in the second.  Child-pair reductions (s = m_c0 + m_c1) are then
packed-half adds (DVE 2x fp16) that depend on ONE block, so level d can start
as soon as the first block of level d+1 is done (cross-level pipelining).
The h~ preact reads child rm halves directly as two PSUM-accumulated matmuls
(no arm tensor at all).

Sharding: data-parallel over trees, 8 trees per core (replicated tables and
weights).  Within a core, 2 groups of 4 trees are emitted in lockstep
(alternating per instruction) so each engine's in-order queue interleaves two
independent dependency chains.
"""

import os
import sys

import numpy as np

for _p in ("/opt/trn_rl_repo",):
    if os.path.isdir(_p) and _p not in sys.path:
        sys.path.insert(0, _p)

B, DEPTH, H, VOCAB = 64, 10, 128, 780
NPT = 2 ** (DEPTH + 1) - 1
NCORES = 8
T = B // NCORES          # trees per core
NGROUPS = 2
TG = T // NGROUPS        # trees per group
VPAD = 1024              # table rows padded (pad idx 0 gathers row 0, unused)

NW = {d: TG * (1 << d) for d in range(DEPTH + 1)}  # per-group level widths
SMALL_D = 6              # d <= SMALL_D: fused small-level path
MM = 512                 # matmul moving-dim max (one PSUM bank)

# K[d]: parent-chunk size of level d; level d+1 blocks are 2*K[d] wide
K = {d: (NW[d] if NW[d] <= 256 else min(1024, NW[d] // 2)) for d in range(DEPTH)}
BLK = {d: 2 * K[d - 1] for d in range(1, DEPTH + 1)}
CW = {d: min(1024, NW[d]) for d in range(DEPTH + 1)}   # compute chunk

# combined small-level x gather: levels SMALL_D..0 in one dma_gather per group
SM_OFF = {}
_o = 0
for _d in range(SMALL_D, -1, -1):
    SM_OFF[_d] = _o
    _o += NW[_d]
SM_COLS = _o
SM_PAD = -(-SM_COLS // 128) * 128

W_NAMES = ("wz1", "wz2", "wh1", "wh2", "wr", "ur", "wg1", "wg2", "wh1d", "wh2d")
B_NAMES = ("bz", "bh", "br", "bg")
NW_ = len(W_NAMES)
BROW_OFF = NW_ * H + len(B_NAMES)
WCOLS = BROW_OFF + 2 * H + 512

# gidx segments, in gather-issue (DMA priority) order.  Names:
#   leaf:<c0>  -> mtab rows for leaf cols [c0, c0+cnt)
#   x<d>:<c0>  -> xtab rows for level-d cols [c0, c0+cnt)
#   small      -> xtab rows for levels SMALL_D..0 (one padded segment)
# The first leaf r-piece of each group needs m[0:1024] and x9[0:1024], so
# those lead the queue at 1024 granularity.
SEG_LIST = []
for _g in range(NGROUPS):
    SEG_LIST.append((_g, "leaf:0", 1024))
    SEG_LIST.append((_g, "x9:0", 1024))
for _g in range(NGROUPS):
    SEG_LIST.append((_g, "leaf:1024", 1024))
    SEG_LIST.append((_g, "x9:1024", 1024))
for _g in range(NGROUPS):
    SEG_LIST.append((_g, "leaf:2048", BLK[DEPTH]))
for _g in range(NGROUPS):
    for _d in range(8, SMALL_D, -1):
        SEG_LIST.append((_g, f"x{_d}:0", NW[_d]))
    SEG_LIST.append((_g, "small", SM_PAD))
SEG_OFF = {}
_c = 0
for _g, _nm, _cnt in SEG_LIST:
    SEG_OFF[(_g, _nm)] = _c
    _c += _cnt // 16
GIDX_COLS = _c

_NC_CACHE = {}


def _build_nc():
    from contextlib import ExitStack

    import concourse.bass as bass
    import concourse.mybir as mybir
    import concourse.tile as tile
    from concourse import bacc

    f16 = mybir.dt.float16
    f32 = mybir.dt.float32
    AF = mybir.ActivationFunctionType
    ALU = mybir.AluOpType

    nc = bacc.Bacc("TRN2", target_bir_lowering=False)

    mtab_d = nc.dram_tensor("mtab", [VPAD, H], f16, kind="ExternalInput")
    xtab_d = nc.dram_tensor("xtab", [VPAD, H], f16, kind="ExternalInput")
    gidx_d = nc.dram_tensor("gidx", [128, GIDX_COLS], mybir.dt.int16, kind="ExternalInput")
    wpack_d = nc.dram_tensor("wpack", [H, WCOLS], f16, kind="ExternalInput")
    out_d = nc.dram_tensor("out", [H, T], f32, kind="ExternalOutput")

    G = list(range(NGROUPS))

    with tile.TileContext(nc) as tc, ExitStack() as ctx:
        consts = ctx.enter_context(tc.tile_pool(name="consts", bufs=1))
        sb = ctx.enter_context(tc.tile_pool(name="sb", bufs=1))
        scr = ctx.enter_context(tc.tile_pool(name="scr", bufs=3))
        pp = ctx.enter_context(tc.tile_pool(name="pp", bufs=2, space="PSUM"))
        pr = ctx.enter_context(tc.tile_pool(name="pr", bufs=2, space="PSUM"))

        gidx = consts.tile([128, GIDX_COLS], mybir.dt.int16, tag="gidx", name="gidx")
        nc.sync.dma_start(out=gidx[:], in_=gidx_d[:])
        wld = consts.tile([H, WCOLS], f16, tag="wld", name="wld")
        nc.sync.dma_start(out=wld[:], in_=wpack_d[:])
        wsb = {n: wld[:, i * H : (i + 1) * H] for i, n in enumerate(W_NAMES)}
        bsb = {n: wld[:, NW_ * H + i : NW_ * H + i + 1] for i, n in enumerate(B_NAMES)}
        brow_bz = wld[:1, BROW_OFF : BROW_OFF + H]
        brow_bh2 = wld[:1, BROW_OFF + H : BROW_OFF + 2 * H]
        ones = wld[:1, BROW_OFF + 2 * H : BROW_OFF + 2 * H + 512]
        warm = consts.tile([H, 1], f32, tag="warm", name="warm")
        nc.scalar.activation(warm[:], wld[:, :1], AF.Sigmoid)

        # ---- persistent per-(group, level) tiles ----
        Xs, Ms, RMs, Ss, XSF = {}, {}, {}, {}, {}
        for g in G:
            for d in range(DEPTH, 0, -1):
                Ms[(g, d)] = sb.tile([128, NW[d]], f16, tag=f"m{g}_{d}", name=f"m{g}_{d}")
                if d > 1:
                    RMs[(g, d)] = sb.tile([128, NW[d]], f16, tag=f"rm{g}_{d}", name=f"rm{g}_{d}")
            for d in range(DEPTH - 1, SMALL_D, -1):
                Xs[(g, d)] = sb.tile([128, NW[d]], f16, tag=f"x{g}_{d}", name=f"x{g}_{d}")
            for d in range(DEPTH - 1, -1, -1):
                Ss[(g, d)] = sb.tile([128, NW[d]], f16, tag=f"s{g}_{d}", name=f"s{g}_{d}")
            XSF[g] = sb.tile([128, SM_PAD], f16, tag=f"xs{g}", name=f"xs{g}")
            for d in range(SMALL_D, -1, -1):
                Xs[(g, d)] = XSF[g][:, SM_OFF[d] : SM_OFF[d] + NW[d]]

        def stile(tag):
            return scr.tile([128, 1024], f16, tag=tag, name=tag)

        def tgather(dst_ap, tab, icol, cnt):
            # HW limit: >512 idxs per transposed dma_gather crashes the exec
            # unit (NRT_EXEC_UNIT_UNRECOVERABLE); the cost model charges the
            # same either way, so split.
            for i0 in range(0, cnt, 512):
                w = min(512, cnt - i0)
                nc.gpsimd.dma_gather(
                    dst_ap[:, i0 : i0 + w].rearrange("p (o n) -> p o n", o=1),
                    tab[:, :],
                    gidx[:, icol + i0 // 16 : icol + (i0 + w) // 16],
                    num_idxs=w,
                    num_idxs_reg=w,
                    elem_size=H,
                    transpose=True,
                )

        def mm(out, wn, mv, start, stop):
            nc.tensor.matmul(out, wsb[wn], mv, start=start, stop=stop)

        def emit_gathers():
            for g, nm, cnt in SEG_LIST:
                if nm.startswith("leaf"):
                    b = int(nm[4:])
                    dst = Ms[(g, DEPTH)][:, b * BLK[DEPTH] : (b + 1) * BLK[DEPTH]]
                    tgather(dst, mtab_d, SEG_OFF[(g, nm)], cnt)
                elif nm.startswith("x"):
                    d = int(nm[1:])
                    tgather(Xs[(g, d)][:, :], xtab_d, SEG_OFF[(g, nm)], cnt)
                else:
                    tgather(XSF[g][:, :], xtab_d, SEG_OFF[(g, nm)], cnt)

        def compute_chunk(g, d, c0, w):
            # zt / sigmoid / htp / tanh / lerp for level-d cols [c0, c0+w)
            X, S, M = Xs[(g, d)], Ss[(g, d)], Ms[(g, d)]
            RMc = RMs[(g, d + 1)]
            cs = slice(c0, c0 + w)
            zt = pp.tile([128, 1024], f32, tag="pp", name=f"zt{g}{d}")
            for q0 in range(0, w, MM):
                qw = min(MM, w - q0)
                mm(zt[:, q0 : q0 + qw], "wz1", X[:, c0 + q0 : c0 + q0 + qw], True, False)
                mm(zt[:, q0 : q0 + qw], "wz2", S[:, c0 + q0 : c0 + q0 + qw], False, True)
            Z = stile("z")
            nc.scalar.activation(Z[:, :w], zt[:, :w], AF.Sigmoid, bias=bsb["bz"])
            ht = pp.tile([128, 1024], f32, tag="pp", name=f"ht{g}{d}")
            k = K[d]
            for s0 in range(c0, c0 + w, k):           # parent sub-chunks
                for q0 in range(0, k, MM):            # 512-col PSUM regions
                    qw = min(MM, k - q0)
                    o = s0 - c0 + q0
                    mm(ht[:, o : o + qw], "wh1", X[:, s0 + q0 : s0 + q0 + qw], True, False)
                    mm(ht[:, o : o + qw], "wh2",
                       RMc[:, 2 * s0 + q0 : 2 * s0 + q0 + qw], False, False)
                    mm(ht[:, o : o + qw], "wh2",
                       RMc[:, 2 * s0 + k + q0 : 2 * s0 + k + q0 + qw], False, True)
            Ht = stile("h")
            nc.scalar.activation(Ht[:, :w], ht[:, :w], AF.Tanh, bias=bsb["bh"])
            U, V = stile("u"), stile("v")
            nc.vector.tensor_sub(U[:, :w], Ht[:, :w], S[:, cs])
            nc.vector.tensor_mul(V[:, :w], Z[:, :w], U[:, :w])
            nc.vector.tensor_add(M[:, cs], S[:, cs], V[:, :w])

        def r_block(g, d, b):
            # S~ of level d-1 for this block, then r/rm of level-d block b
            W = BLK[d]
            kp = W // 2
            b0 = b * W
            M = Ms[(g, d)]
            nc.vector.tensor_add(
                Ss[(g, d - 1)][:, b * kp : (b + 1) * kp],
                M[:, b0 : b0 + kp], M[:, b0 + kp : b0 + W],
            )
            if d == 1:
                return
            xp = Xs[(g, d - 1)][:, b * kp : (b + 1) * kp]
            sub = min(kp, MM)
            for p0 in range(0, W, 1024):              # PSUM pieces
                pw = min(1024, W - p0)
                rt = pr.tile([128, 1024], f32, tag="pr", name=f"rt{g}{d}")
                for q0 in range(0, pw, MM):           # one accum group per bank
                    qw = min(MM, pw - q0)
                    mm(rt[:, q0 : q0 + qw], "ur",
                       M[:, b0 + p0 + q0 : b0 + p0 + q0 + qw], True, False)
                    for s0 in range(q0, q0 + qw, sub):
                        sw = min(sub, q0 + qw - s0)
                        # parity half of col (p0+s0) within the block
                        xq = (p0 + s0) % kp
                        mm(rt[:, s0 : s0 + sw], "wr", xp[:, xq : xq + sw],
                           False, s0 + sw == q0 + qw)
                R = stile("r")
                nc.scalar.activation(R[:, :pw], rt[:, :pw], AF.Sigmoid, bias=bsb["br"])
                nc.vector.tensor_mul(
                    RMs[(g, d)][:, b0 + p0 : b0 + p0 + pw], R[:, :pw],
                    M[:, b0 + p0 : b0 + p0 + pw],
                )

        def small_level(g, d):
            # one block; fused z|h sigmoid (bias-fold); rt = Ur@S + Ur@v + Wr@xp
            n = NW[d]
            X, S, M = Xs[(g, d)], Ss[(g, d)], Ms[(g, d)]
            RMc = RMs[(g, d + 1)]
            zh = pp.tile([128, 1024], f32, tag="pp", name=f"zh{g}{d}")
            nc.tensor.matmul(zh[:, :n], brow_bz, ones[:, :n], start=True, stop=False)
            mm(zh[:, :n], "wz1", X[:, :n], False, False)
            mm(zh[:, :n], "wz2", S[:, :n], False, True)
            nc.tensor.matmul(zh[:, n : 2 * n], brow_bh2, ones[:, :n], start=True, stop=False)
            mm(zh[:, n : 2 * n], "wh1d", X[:, :n], False, False)
            mm(zh[:, n : 2 * n], "wh2d", RMc[:, :n], False, False)
            mm(zh[:, n : 2 * n], "wh2d", RMc[:, n : 2 * n], False, True)
            rt = None
            if d > 1:
                rt = pr.tile([128, 1024], f32, tag="pr", name=f"rts{g}{d}")
                mm(rt[:, :n], "ur", S[:, :n], True, False)
            Z = stile("z")
            nc.scalar.activation(Z[:, : 2 * n], zh[:, : 2 * n], AF.Sigmoid)
            z, sh = Z[:, :n], Z[:, n : 2 * n]
            U, V = stile("u"), stile("v")
            nc.vector.tensor_scalar(out=U[:, :n], in0=sh, scalar1=2.0, scalar2=-1.0,
                                    op0=ALU.mult, op1=ALU.add)
            nc.vector.tensor_sub(U[:, :n], U[:, :n], S[:, :n])
            nc.vector.tensor_mul(V[:, :n], z, U[:, :n])
            nc.vector.tensor_add(M[:, :n], S[:, :n], V[:, :n])
            np_ = n // 2
            nc.vector.tensor_add(Ss[(g, d - 1)][:, :np_], M[:, :np_], M[:, np_:n])
            if d == 1:
                return
            xpb = (
                Xs[(g, d - 1)][:, :np_]
                .rearrange("p (o n) -> p o n", o=1)
                .to_broadcast([128, 2, np_])
            )
            mm(rt[:, :n], "ur", V[:, :n], False, False)
            nc.tensor.matmul(rt[:, :n], wsb["wr"], xpb, start=False, stop=True)
            R = stile("r")
            nc.scalar.activation(R[:, :n], rt[:, :n], AF.Sigmoid, bias=bsb["br"])
            nc.vector.tensor_mul(RMs[(g, d)][:, :n], R[:, :n], M[:, :n])

        def root_readout(g):
            S0, X0 = Ss[(g, 0)], Xs[(g, 0)]
            pg = pr.tile([128, 1024], f32, tag="pr", name=f"pg{g}")
            mm(pg[:, :TG], "wg1", X0[:, :TG], True, False)
            mm(pg[:, :TG], "wg2", S0[:, :TG], False, True)
            outt = sb.tile([128, TG], f32, tag=f"out{g}", name=f"out{g}")
            nc.scalar.activation(outt[:], pg[:, :TG], AF.Relu, bias=bsb["bg"])
            nc.sync.dma_start(out=out_d[:, g * TG : (g + 1) * TG], in_=outt[:])

        # ---------------- schedule ----------------
        emit_gathers()
        for b in range(NW[DEPTH] // BLK[DEPTH]):      # leaf r/rm + S~ blocks
            for g in G:
                r_block(g, DEPTH, b)
        for d in range(DEPTH - 1, 0, -1):
            if d > SMALL_D:
                n = NW[d]
                for c0 in range(0, n, CW[d]):
                    for g in G:
                        compute_chunk(g, d, c0, min(CW[d], n - c0))
                for b in range(n // BLK[d]):
                    for g in G:
                        r_block(g, d, b)
            else:
                for g in G:
                    small_level(g, d)
        for g in G:
            root_readout(g)

    nc.finalize()
    return nc


def get_nc():
    if "nc" not in _NC_CACHE:
        _NC_CACHE["nc"] = _build_nc()
    return _NC_CACHE["nc"]


def make_core_inputs(wid, emb, weights):
    wid = np.asarray(wid).reshape(B, NPT)
    embf = np.asarray(emb, dtype=np.float32)

    Wz1 = np.asarray(weights["Wz_w"][:H], np.float32)
    Wz2 = np.asarray(weights["Wz_w"][H:], np.float32)
    Wh1 = np.asarray(weights["Wh_w"][:H], np.float32)
    Wh2 = np.asarray(weights["Wh_w"][H:], np.float32)
    Wr = np.asarray(weights["Wr_w"], np.float32)
    Ur = np.asarray(weights["Ur_w"], np.float32)
    Wg1 = np.asarray(weights["Wg_w"][:H], np.float32)
    Wg2 = np.asarray(weights["Wg_w"][H:], np.float32)
    bz = np.asarray(weights["Wz_b"], np.float32)
    bh = np.asarray(weights["Wh_b"], np.float32)
    br = np.asarray(weights["Ur_b"], np.float32)
    bg = np.asarray(weights["Wg_b"], np.float32)

    zt = embf @ Wz1 + bz
    ht = embf @ Wh1 + bh
    mtab = (1.0 / (1.0 + np.exp(-zt))) * np.tanh(ht)
    mtab_p = np.zeros((VPAD, H), np.float32)
    mtab_p[:VOCAB] = mtab
    xtab_p = np.zeros((VPAD, H), np.float32)
    xtab_p[:VOCAB] = embf

    wpack = np.zeros((H, WCOLS), np.float32)
    mats = {"wz1": Wz1, "wz2": Wz2, "wh1": Wh1, "wh2": Wh2, "wr": Wr, "ur": Ur,
            "wg1": Wg1, "wg2": Wg2, "wh1d": 2 * Wh1, "wh2d": 2 * Wh2}
    for i, nname in enumerate(W_NAMES):
        wpack[:, i * H : (i + 1) * H] = mats[nname]
    bcols = {"bz": bz, "bh": bh, "br": br, "bg": bg}
    for i, nname in enumerate(B_NAMES):
        wpack[:, NW_ * H + i] = bcols[nname]
    wpack[0, BROW_OFF : BROW_OFF + H] = bz
    wpack[0, BROW_OFF + H : BROW_OFF + 2 * H] = 2 * bh
    wpack[0, BROW_OFF + 2 * H : BROW_OFF + 2 * H + 512] = 1.0

    # per-group block-local parity orders (flat ids t * NPT + h)
    order = {0: [t * NPT + 0 for t in range(TG)]}
    for d in range(1, DEPTH + 1):
        prev = order[d - 1]
        k = K[d - 1]
        nxt = []
        for c0 in range(0, len(prev), k):
            ch = prev[c0 : c0 + k]
            nxt += [f + (f % NPT) + 1 for f in ch]
            nxt += [f + (f % NPT) + 2 for f in ch]
        order[d] = nxt
    ORD = {d: np.asarray(order[d], np.int64) for d in order}

    in_maps = []
    base = {
        "mtab": np.ascontiguousarray(mtab_p.astype(np.float16)),
        "xtab": np.ascontiguousarray(xtab_p.astype(np.float16)),
        "wpack": np.ascontiguousarray(wpack.astype(np.float16)),
    }
    for c in range(NCORES):
        widc = wid[c * T : (c + 1) * T]  # [T, NPT]
        seg_ids = {}
        for g in range(NGROUPS):
            trees = widc[g * TG : (g + 1) * TG].reshape(-1)  # flat [TG * NPT]
            lv = {d: trees[ORD[d]] for d in range(DEPTH + 1)}
            for b in range(NW[DEPTH] // BLK[DEPTH]):
                seg_ids[(g, f"leaf{b}")] = lv[DEPTH][b * BLK[DEPTH] : (b + 1) * BLK[DEPTH]]
            for d in range(DEPTH - 1, SMALL_D, -1):
                seg_ids[(g, f"x{d}")] = lv[d]
            sm = np.concatenate([lv[d] for d in range(SMALL_D, -1, -1)])
            seg_ids[(g, "small")] = np.concatenate(
                [sm, np.zeros(SM_PAD - len(sm), np.int64)]
            )
        ids = np.concatenate([seg_ids[(g, nm)] for g, nm, _ in SEG_LIST])
        ids = ids.astype(np.int16)
        gi = ids.reshape(-1, 16).T  # [16, GIDX_COLS]
        in_maps.append({**base, "gidx": np.ascontiguousarray(np.tile(gi, (8, 1)))})
    return in_maps


def kernel(**inputs):
    from concourse.bass_utils import run_bass_kernel_spmd

    nc = get_nc()
    in_maps = make_core_inputs(inputs["wid"], inputs["emb"], inputs)
    res = run_bass_kernel_spmd(nc, in_maps, core_ids=list(range(NCORES)))
    out = np.concatenate(
        [np.asarray(res.results[c]["out"]).T for c in range(NCORES)], axis=0
    )
    return np.ascontiguousarray(out.astype(np.float32))
